# revision 11
# baseline (speedup 1.0000x reference)
"""Trainium2 Bass kernel for nn_BitwiseMultipyLogis (gnn_message_passing).

Reference computation (L=8 layers, N=100000 nodes, F=128 features):
    proj    = tanh(node_features @ trans + bias)          # [L, N, F]
    bitwise = proj * proj[layer_predict]                  # [L, N, F]
    bitwise = einsum('lnf,lfg->lng', bitwise, theta)      # [L, N, F]
    scores  = sigmoid(bitwise @ logis_w[0] + logis_b)     # [L, N]
    weights = softmax(scores, axis=0)                     # [L, N]
    out     = proj[layer_predict] + sum_l weights[l]*proj[l]   # [N, F]

Key algebraic simplification: theta only feeds the logis_w dot product, so
    scores[l,n] = sigmoid( sum_f proj[l,n,f]*proj[lp,n,f]*v[l,f] + logis_b )
with v[l] = theta[l] @ logis_w[0] precomputed on host.  This removes the
entire [L,N,F]x[L,F,F] einsum (half the FLOPs).

Wall-clock structure (measured): the axon tunnel moves ~0.1 GB/s in and
~0.03 GB/s out with ~140ms/call latency, and the host has ONE cpu core.
So the run is dominated by host prep + transfer, not device compute:
  * input stays node-major [L, nodes, F] bf16 (host does ONLY an
    astype+block-copy, ~0.2s); the device transposes tiles for free with
    dma_start_transpose (XBAR) during the load.
  * the one-hot v8 score matrix is built on device from a tiny [128, 8]
    table, so per-call constants are ~35KB instead of ~300KB.
  * output is quantized on device to int8 (scale 63.5, |out| <= 2) so
    readback is 13MB instead of 51MB f32.
  * the jitted SPMD executable is built once and cached; output zero
    buffers are created inside the jit (no per-call zero upload).

Device pipeline per [128f, 512n] tile (per core, 25 tiles, data-parallel
over nodes, 12500+300pad nodes/core):
  * xT via dma_start_transpose; projT = tanh(trans^T @ xT) on TensorE
    (bf16, f32 PSUM) + ScalarE.
  * scores via accumulated matmuls with one-hot-masked v columns; layer
    l's score row lands at partition 32*(l%3) of score group l//3.
  * sigmoid+softmax without table swap: sigmoid(x)=(1+tanh(x/2))/2 and
    exp(sigmoid(x)) = exp(0.5*tanh(x/2) + 0.5); max-subtraction safe to
    skip since sigmoid outputs are in (0,1).
  * softmax denominator via ones matmul; weights broadcast across the
    128 partitions with K=1 matmuls; weighted sum accumulated in PSUM
    via identity matmuls; final add + int8 quant, node transpose on host
    (output is small).
"""

import numpy as np

import concourse.bass as bass
import concourse.mybir as mybir
import concourse.tile as tile
from concourse import bacc

DT16 = mybir.dt.float16
F32 = mybir.dt.float32
I8 = mybir.dt.int8
AF = mybir.ActivationFunctionType

L, N, F = 8, 100000, 128
CORES = 8
NS = N // CORES            # 12500 nodes per core
TILE = 512                 # node columns per tile (one f32 PSUM bank)
NT = (NS + TILE - 1) // TILE   # 25
NSP = NT * TILE            # 12800 (padded)
OSCALE = 63.5              # int8 output scale; |out| <= 2 so |q| <= 127

NP16 = np.float16


def _body(tc, out, ins, lp: int, logis_b: float, nt: int):
    """Emit the tile program.  out: [128, nt*TILE] int8 dram AP;
    ins: dict of dram APs (xt node-major [L, nt*TILE, 128])."""
    from contextlib import ExitStack
    nc = tc.nc
    with ExitStack() as ctx:
        const = ctx.enter_context(tc.tile_pool(name="const", bufs=1))
        xts = ctx.enter_context(tc.tile_pool(name="xts", bufs=2))
        projp = ctx.enter_context(tc.tile_pool(name="projp", bufs=2, space="PSUM"))
        projs = ctx.enter_context(tc.tile_pool(name="projs", bufs=2))
        bits = ctx.enter_context(tc.tile_pool(name="bits", bufs=2))
        scp = ctx.enter_context(tc.tile_pool(name="scp", bufs=1, space="PSUM"))
        scs = ctx.enter_context(tc.tile_pool(name="scs", bufs=2))
        wbp = ctx.enter_context(tc.tile_pool(name="wbp", bufs=2, space="PSUM"))
        ys = ctx.enter_context(tc.tile_pool(name="ys", bufs=2))
        sump = ctx.enter_context(tc.tile_pool(name="sump", bufs=1, space="PSUM"))
        outs = ctx.enter_context(tc.tile_pool(name="outs", bufs=2))

        trans_sb = const.tile([128, 128], DT16)
        nc.sync.dma_start(trans_sb[:], ins["trans"])
        # v8sp: per layer l a [128, 128] one-hot-column matrix whose column
        # 32*(l%3) holds v[l]; used as lhsT so layer l's score row lands at
        # partition 32*(l%3) of score group l//3 (base partitions are limited
        # to {0,32,64} for later rhs reads, so 3 layers per PSUM bank).
        # Built on device from the dense [128, L] v8c table.
        v8c_sb = const.tile([128, L], DT16)
        nc.sync.dma_start(v8c_sb[:], ins["v8c"])
        v8sp_sb = const.tile([128, L * 128], DT16)
        nc.gpsimd.memset(v8sp_sb[:], 0.0)
        for l in range(L):
            col = l * 128 + 32 * (l % 3)
            nc.vector.tensor_copy(v8sp_sb[:, col:col + 1], v8c_sb[:, l:l + 1])
        ident_sb = const.tile([128, 128], DT16)
        nc.sync.dma_start(ident_sb[:], ins["ident"])
        # selection columns: col0 = ones at {0,32,64}, col1 = ones at {0,32}
        sel32_sb = const.tile([128, 2], F32)
        nc.sync.dma_start(sel32_sb[:], ins["sel32"])
        # all-ones rows: K=1 lhsT that replicates a [1, n] rhs row across
        # all 128 output partitions (PE-based partition broadcast).
        onesr32_sb = const.tile([128, 128], F32)
        nc.sync.dma_start(onesr32_sb[:], ins["onesr32"])
        bias_sb = const.tile([128, 1], F32)
        nc.sync.dma_start(bias_sb[:], ins["biasc"])
        lb_bias = const.tile([128, 1], F32)
        nc.gpsimd.memset(lb_bias[:], 0.5 * logis_b)
        half_bias = const.tile([128, 1], F32)
        nc.gpsimd.memset(half_bias[:], 0.5)

        xt = ins["xt"]
        for t in range(nt):
            # transposing loads: [512n, 128f] dram -> [128f, 512n] sbuf
            xt_sb = xts.tile([128, L, TILE], DT16, tag="xt")
            for l in range(L):
                nc.sync.dma_start_transpose(
                    xt_sb[:, l, :], xt[l, t * TILE:(t + 1) * TILE, :])

            # projT[l] = tanh(trans^T @ xT[l] + bias)   [128f, TILE]
            proj = projs.tile([128, L, TILE], DT16, tag="proj")
            for l in range(L):
                pp = projp.tile([128, TILE], F32, tag="pp")
                nc.tensor.matmul(pp[:], trans_sb[:], xt_sb[:, l, :],
                                 start=True, stop=True)
                nc.scalar.activation(proj[:, l, :], pp[:], AF.Tanh,
                                     bias=bias_sb[:, 0:1], scale=1.0)

            # bit[l] = projT[l] * projT[lp]
            bit = bits.tile([128, L, TILE], DT16, tag="bit")
            for l in range(L):
                nc.vector.tensor_mul(bit[:, l, :], proj[:, l, :], proj[:, lp, :])

            # scores_raw[l, n] = sum_f v[l,f] * bit[l,f,n].  Layer l's score
            # row lands at partition 32*(l%3) of score group l//3: groups 0/1
            # in the two banks of sc_psA, group 2 (layers 6,7) in sc_psB.
            expvs = []
            for g in range(3):
                nls = 3 if g < 2 else 2
                m = 32 * (nls - 1) + 1
                sc_ps = scp.tile([128, TILE], F32, tag=f"scps{g}")
                for s in range(nls):
                    l = 3 * g + s
                    nc.tensor.matmul(
                        sc_ps[0:m, :],
                        v8sp_sb[:, l * 128: l * 128 + m],
                        bit[:, l, :],
                        start=(s == 0), stop=(s == nls - 1),
                    )
                # e = exp(sigmoid(raw + lb)) with no table swap:
                # t = tanh(0.5*raw + 0.5*lb); e = exp(0.5*t + 0.5)
                sct = scs.tile([128, TILE], F32, tag=f"sct{g}")
                nc.scalar.activation(sct[0:m, :], sc_ps[0:m, :], AF.Tanh,
                                     bias=lb_bias[0:m, :], scale=0.5)
                expv = scs.tile([128, TILE], F32, tag=f"expv{g}")
                nc.scalar.activation(expv[0:m, :], sct[0:m, :], AF.Exp,
                                     bias=half_bias[0:m, :], scale=0.5)
                expvs.append(expv)

            def _erow(l):
                g, s = divmod(l, 3)
                return expvs[g][32 * s: 32 * s + 1, :]

            # sumexp + reciprocal
            se_ps = sump.tile([1, TILE], F32, tag="seps")
            nc.tensor.matmul(se_ps[:], sel32_sb[0:65, 0:1], expvs[0][0:65, :],
                             start=True, stop=False)
            nc.tensor.matmul(se_ps[:], sel32_sb[0:65, 0:1], expvs[1][0:65, :],
                             start=False, stop=False)
            nc.tensor.matmul(se_ps[:], sel32_sb[0:33, 1:2], expvs[2][0:33, :],
                             start=False, stop=True)
            rec = scs.tile([1, TILE], F32, tag="rec")
            nc.vector.reciprocal(rec[:], se_ps[:])

            # y[l] = projT[l] * e_bcast[l];  agg = sum_l y[l]  (identity MMs).
            y = ys.tile([128, L, TILE], DT16, tag="y")
            for l in range(L):
                wb = wbp.tile([128, TILE], F32, tag="wagg")
                q = 32 * (l % 3)
                nc.tensor.matmul(wb[:], onesr32_sb[q: q + 1, :], _erow(l),
                                 start=True, stop=True)
                nc.vector.tensor_mul(y[:, l, :], proj[:, l, :], wb[:])
            agg = wbp.tile([128, TILE], F32, tag="wagg")
            for l in range(L):
                nc.tensor.matmul(agg[:], ident_sb[:], y[:, l, :],
                                 start=(l == 0), stop=(l == L - 1))

            # out_q = round(63.5 * (projT[lp] + agg * recip_bcast))  int8
            rb = wbp.tile([128, TILE], F32, tag="wagg")
            nc.tensor.matmul(rb[:], onesr32_sb[0:1, :], rec[:],
                             start=True, stop=True)
            rb_sb = outs.tile([128, TILE], F32, tag="rbsb")
            nc.vector.tensor_copy(rb_sb[:], rb[:])
            nrm = outs.tile([128, TILE], F32, tag="nrm")
            nc.vector.tensor_mul(nrm[:], agg[:], rb_sb[:])
            ot = outs.tile([128, TILE], F32, tag="ot")
            nc.vector.tensor_add(ot[:], nrm[:], proj[:, lp, :])
            oq = outs.tile([128, TILE], I8, tag="oq")
            nc.scalar.activation(oq[:], ot[:], AF.Copy, bias=0.0, scale=OSCALE)
            nc.sync.dma_start(out[:, t * TILE:(t + 1) * TILE], oq[:])


def _build(lp: int, logis_b: float, nt: int = NT):
    nc = bacc.Bacc("TRN2", target_bir_lowering=False, debug=False,
                   num_devices=CORES)
    ins = {
        "xt": nc.dram_tensor("xt", [L, nt * TILE, 128], DT16,
                             kind="ExternalInput").ap(),
        "trans": nc.dram_tensor("trans", [128, 128], DT16,
                                kind="ExternalInput").ap(),
        "v8c": nc.dram_tensor("v8c", [128, L], DT16,
                              kind="ExternalInput").ap(),
        "ident": nc.dram_tensor("ident", [128, 128], DT16,
                                kind="ExternalInput").ap(),
        "sel32": nc.dram_tensor("sel32", [128, 2], F32,
                                kind="ExternalInput").ap(),
        "onesr32": nc.dram_tensor("onesr32", [128, 128], F32,
                                  kind="ExternalInput").ap(),
        "biasc": nc.dram_tensor("biasc", [128, 1], F32,
                                kind="ExternalInput").ap(),
    }
    out = nc.dram_tensor("out", [128, nt * TILE], I8,
                         kind="ExternalOutput").ap()
    with tile.TileContext(nc) as tc:
        _body(tc, out, ins, lp, logis_b, nt)
    nc.compile()
    return nc


# ---------------------------------------------------------------- host side

def _host_prep(inputs):
    """Returns (x_global bf16 [CORES*L, NSP, 128], per-call consts dict, lp, lb).
    The global arrays are concatenated along axis 0 (shard_map convention)."""
    nf = np.asarray(inputs["node_features"], np.float32)      # [L, N, F]
    trans = np.asarray(inputs["trans"], np.float32)           # [F, F]
    biasv = np.asarray(inputs["bias"], np.float32).reshape(F)
    theta = np.asarray(inputs["theta"], np.float32)           # [L, F, F]
    lw = np.asarray(inputs["logis_w"], np.float32).reshape(1, F)
    lb = float(np.asarray(inputs["logis_b"], np.float32).reshape(-1)[0])
    lp = int(np.asarray(inputs["layer_predict"]).reshape(-1)[0])

    # node-major blocked copy + bf16 conversion (single pass, ~0.2s);
    # np.zeros gives zero pad pages for free.
    xg = np.zeros((CORES * L, NSP, F), dtype=NP16)
    for c in range(CORES):
        for l in range(L):
            xg[c * L + l, :NS] = nf[l, c * NS:(c + 1) * NS]

    v8 = theta @ lw[0]                                        # [L, F]
    consts = {
        "trans": np.tile(trans.astype(NP16), (CORES, 1)),
        "v8c": np.tile(np.ascontiguousarray(v8.T).astype(NP16), (CORES, 1)),
        "biasc": np.tile(biasv.reshape(128, 1), (CORES, 1)),
    }
    return xg, consts, lp, lb


def _fixed_consts():
    """Input-independent constants (device-cached after first call)."""
    sel32 = np.zeros((128, 2), np.float32)
    sel32[[0, 32, 64], 0] = 1.0
    sel32[[0, 32], 1] = 1.0
    return {
        "ident": np.tile(np.eye(128, dtype=np.float32).astype(NP16), (CORES, 1)),
        "sel32": np.tile(sel32, (CORES, 1)),
        "onesr32": np.tile(np.ones((128, 128), np.float32), (CORES, 1)),
    }


# ------------------------------------------------------------------- runner

_STATE = {}


def _get_state(lp: int, lb: float):
    key = (lp, round(lb, 8))
    if key in _STATE:
        return _STATE[key]

    import jax
    import jax.numpy as jnp
    from jax.sharding import Mesh, PartitionSpec, NamedSharding
    from jax.experimental.shard_map import shard_map
    import concourse.bass2jax as b2j
    from concourse import mybir as _mb

    b2j.install_neuronx_cc_hook()
    nc = _build(lp, lb)

    in_names, out_names, out_avals = [], [], []
    for alloc in nc.m.functions[0].allocations:
        if not isinstance(alloc, _mb.MemoryLocationSet):
            continue
        name = alloc.memorylocations[0].name
        if alloc.kind == "ExternalInput":
            in_names.append(name)
        elif alloc.kind == "ExternalOutput":
            out_names.append(name)
            out_avals.append(jax.core.ShapedArray(
                tuple(alloc.tensor_shape), _mb.dt.np(alloc.dtype)))

    pid_name = nc.partition_id_tensor.name if nc.partition_id_tensor else None
    if pid_name is not None and pid_name in in_names:
        in_names.remove(pid_name)

    devices = jax.devices()[:CORES]
    mesh = Mesh(np.asarray(devices), ("core",))
    sharding = NamedSharding(mesh, PartitionSpec("core"))

    all_names = tuple(in_names) + tuple(out_names)
    if pid_name is not None:
        all_names = all_names + (pid_name,)

    def _bodyf(*args):
        ops = list(args)
        if pid_name is not None:
            ops.append(b2j.partition_id_tensor())
        outs = b2j._bass_exec_p.bind(
            *ops,
            out_avals=tuple(out_avals),
            in_names=all_names,
            out_names=tuple(out_names),
            lowering_input_output_aliases=(),
            sim_require_finite=True,
            sim_require_nnan=True,
            nc=nc,
        )
        return tuple(outs)

    n_args = len(in_names) + len(out_names)
    f = jax.jit(shard_map(
        _bodyf, mesh=mesh,
        in_specs=(PartitionSpec("core"),) * n_args,
        out_specs=(PartitionSpec("core"),) * len(out_names),
        check_rep=False))

    fixed_dev = {k: jax.device_put(v, sharding)
                 for k, v in _fixed_consts().items()}
    # Phantom "out" parameters: the NEFF tensor rename (in_rename |
    # out_rename) drops the input binding for ExternalOutput names, so the
    # contents are never read — the kernel writes every output element.
    # Device-cached once; NOT donated so they survive across calls.
    out_dummies = [jax.device_put(
        np.zeros((CORES * a.shape[0],) + tuple(a.shape[1:]), a.dtype),
        sharding) for a in out_avals]

    st = {"f": f, "in_names": in_names, "out_names": out_names,
          "sharding": sharding, "fixed_dev": fixed_dev,
          "out_dummies": out_dummies, "nc": nc}
    _STATE[key] = st
    return st


def _run(inputs):
    import jax

    xg, consts, lp, lb = _host_prep(inputs)
    st = _get_state(lp, lb)

    x_dev = jax.device_put(xg, st["sharding"])
    args = []
    for name in st["in_names"]:
        if name == "xt":
            args.append(x_dev)
        elif name in consts:
            args.append(consts[name])
        else:
            args.append(st["fixed_dev"][name])
    args.extend(st["out_dummies"])
    out = st["f"](*args)
    q = np.asarray(out[0]).reshape(CORES, 128, NSP)           # int8

    full = np.empty((N, F), np.float32)
    for c in range(CORES):
        full[c * NS:(c + 1) * NS] = q[c, :, :NS].T.astype(np.float32)
    full *= np.float32(1.0 / OSCALE)
    return full


def kernel(**inputs) -> np.ndarray:
    return _run(inputs)


# revision 15
# speedup vs baseline: 1.5109x; 1.5109x over previous
"""Trainium2 Bass kernel for nn_BitwiseMultipyLogis (gnn_message_passing).

Reference computation (L=8 layers, N=100000 nodes, F=128 features):
    proj    = tanh(node_features @ trans + bias)          # [L, N, F]
    bitwise = proj * proj[layer_predict]                  # [L, N, F]
    bitwise = einsum('lnf,lfg->lng', bitwise, theta)      # [L, N, F]
    scores  = sigmoid(bitwise @ logis_w[0] + logis_b)     # [L, N]
    weights = softmax(scores, axis=0)                     # [L, N]
    out     = proj[layer_predict] + sum_l weights[l]*proj[l]   # [N, F]

Key algebraic simplification: theta only feeds the logis_w dot product, so
    scores[l,n] = sigmoid( sum_f proj[l,n,f]*proj[lp,n,f]*v[l,f] + logis_b )
with v[l] = theta[l] @ logis_w[0] precomputed on host.  This removes the
entire [L,N,F]x[L,F,F] einsum (half the FLOPs).

Wall-clock structure (measured): the axon tunnel moves ~0.02-0.1 GB/s
with ~140ms/op latency, and the host has ONE cpu core.  So the run is
dominated by host prep + transfer, not device compute:
  * input stays node-major [L, nodes, F] fp16 (host does ONLY an
    astype+block-copy, ~0.2s); the device transposes tiles during the
    load with dma_start_transpose (XBAR).  fp16 rather than bf16: same
    wire bytes, 8x less input quantization error.
  * the one-hot v8 score matrix is built on device from a tiny [128, 8]
    table, so per-call constants are ~35KB instead of ~300KB.
  * output is quantized on device to int8 (scale 63.5, |out| <= 2) so
    readback is 12.8MB instead of 51MB f32.
  * the jitted SPMD executable is built once and cached; input-
    independent constants and the phantom output parameters are
    device-resident (nothing but x + 3 small tables moves per call).

Device pipeline per [128f, <=512n] tile (per core, 24 full tiles + one
224-wide tail = 12512 cols, data-parallel over nodes, 12500/core):
  * xT via dma_start_transpose; projT = tanh(trans^T @ xT) on TensorE
    (fp16, f32 PSUM) + ScalarE.
  * scores via accumulated matmuls with one-hot-masked v columns; layer
    l's score row lands at partition 32*(l%3) of score group l//3.
  * sigmoid+softmax without table swap: sigmoid(x)=(1+tanh(x/2))/2 and
    exp(sigmoid(x)) = exp(0.5*tanh(x/2) + 0.5); max-subtraction safe to
    skip since sigmoid outputs are in (0,1).
  * softmax denominator via ones matmul; weights broadcast across the
    128 partitions with K=1 matmuls; weighted sum accumulated in PSUM
    via identity matmuls; final add + int8 quant, node transpose on host
    (output is small).
"""

import numpy as np

import concourse.bass as bass
import concourse.mybir as mybir
import concourse.tile as tile
from concourse import bacc

DT16 = mybir.dt.float16
F32 = mybir.dt.float32
I8 = mybir.dt.int8
AF = mybir.ActivationFunctionType

L, N, F = 8, 100000, 128
CORES = 8
NS = N // CORES            # 12500 nodes per core
TILE = 512                 # node columns per tile (one f32 PSUM bank)
# 24 full tiles + one 224-wide tail (224 keeps the XBAR 16-row rule);
# NSP = 12512 pads only 12 nodes per core instead of 300.
TILES = [TILE] * (NS // TILE) + [((NS % TILE) + 15) // 16 * 16]
NT = len(TILES)            # 25
NSP = sum(TILES)           # 12512
OSCALE = 63.5              # int8 output scale; |out| <= 2 so |q| <= 127

NP16 = np.float16


def _body(tc, out, ins, lp: int, logis_b: float, nt: int):
    """Emit the tile program.  out: [128, nt*TILE] int8 dram AP;
    ins: dict of dram APs (xt node-major [L, nt*TILE, 128])."""
    from contextlib import ExitStack
    nc = tc.nc
    with ExitStack() as ctx:
        const = ctx.enter_context(tc.tile_pool(name="const", bufs=1))
        xts = ctx.enter_context(tc.tile_pool(name="xts", bufs=2))
        projp = ctx.enter_context(tc.tile_pool(name="projp", bufs=2, space="PSUM"))
        projs = ctx.enter_context(tc.tile_pool(name="projs", bufs=2))
        bits = ctx.enter_context(tc.tile_pool(name="bits", bufs=2))
        scp = ctx.enter_context(tc.tile_pool(name="scp", bufs=1, space="PSUM"))
        scs = ctx.enter_context(tc.tile_pool(name="scs", bufs=2))
        wbp = ctx.enter_context(tc.tile_pool(name="wbp", bufs=2, space="PSUM"))
        ys = ctx.enter_context(tc.tile_pool(name="ys", bufs=2))
        sump = ctx.enter_context(tc.tile_pool(name="sump", bufs=1, space="PSUM"))
        outs = ctx.enter_context(tc.tile_pool(name="outs", bufs=2))

        trans_sb = const.tile([128, 128], DT16)
        nc.sync.dma_start(trans_sb[:], ins["trans"])
        # v8sp: per layer l a [128, 128] one-hot-column matrix whose column
        # 32*(l%3) holds v[l]; used as lhsT so layer l's score row lands at
        # partition 32*(l%3) of score group l//3 (base partitions are limited
        # to {0,32,64} for later rhs reads, so 3 layers per PSUM bank).
        # Built on device from the dense [128, L] v8c table.
        v8c_sb = const.tile([128, L], DT16)
        nc.sync.dma_start(v8c_sb[:], ins["v8c"])
        v8sp_sb = const.tile([128, L * 128], DT16)
        nc.gpsimd.memset(v8sp_sb[:], 0.0)
        for l in range(L):
            col = l * 128 + 32 * (l % 3)
            nc.vector.tensor_copy(v8sp_sb[:, col:col + 1], v8c_sb[:, l:l + 1])
        ident_sb = const.tile([128, 128], DT16)
        nc.sync.dma_start(ident_sb[:], ins["ident"])
        # selection columns: col0 = ones at {0,32,64}, col1 = ones at {0,32}
        sel32_sb = const.tile([128, 2], F32)
        nc.sync.dma_start(sel32_sb[:], ins["sel32"])
        # all-ones rows: K=1 lhsT that replicates a [1, n] rhs row across
        # all 128 output partitions (PE-based partition broadcast).
        onesr32_sb = const.tile([128, 128], F32)
        nc.sync.dma_start(onesr32_sb[:], ins["onesr32"])
        bias_sb = const.tile([128, 1], F32)
        nc.sync.dma_start(bias_sb[:], ins["biasc"])
        lb_bias = const.tile([128, 1], F32)
        nc.gpsimd.memset(lb_bias[:], 0.5 * logis_b)
        half_bias = const.tile([128, 1], F32)
        nc.gpsimd.memset(half_bias[:], 0.5)

        xt = ins["xt"]
        off = 0
        for t in range(nt):
            w = TILES[t]   # 512, except 224 on the tail tile
            # transposing loads: [w n, 128f] dram -> [128f, w n] sbuf
            xt_sb = xts.tile([128, L, TILE], DT16, tag="xt")
            for l in range(L):
                nc.sync.dma_start_transpose(
                    xt_sb[:, l, 0:w], xt[l, off:off + w, :])

            # projT[l] = tanh(trans^T @ xT[l] + bias)   [128f, w]
            proj = projs.tile([128, L, TILE], DT16, tag="proj")
            for l in range(L):
                pp = projp.tile([128, TILE], F32, tag="pp")
                nc.tensor.matmul(pp[:, 0:w], trans_sb[:], xt_sb[:, l, 0:w],
                                 start=True, stop=True)
                nc.scalar.activation(proj[:, l, 0:w], pp[:, 0:w], AF.Tanh,
                                     bias=bias_sb[:, 0:1], scale=1.0)

            # bit[l] = projT[l] * projT[lp]
            bit = bits.tile([128, L, TILE], DT16, tag="bit")
            for l in range(L):
                nc.vector.tensor_mul(bit[:, l, 0:w], proj[:, l, 0:w],
                                     proj[:, lp, 0:w])

            # scores_raw[l, n] = sum_f v[l,f] * bit[l,f,n].  Layer l's score
            # row lands at partition 32*(l%3) of score group l//3: groups 0/1
            # in the two banks of sc_psA, group 2 (layers 6,7) in sc_psB.
            expvs = []
            for g in range(3):
                nls = 3 if g < 2 else 2
                m = 32 * (nls - 1) + 1
                sc_ps = scp.tile([128, TILE], F32, tag=f"scps{g}")
                for s in range(nls):
                    l = 3 * g + s
                    nc.tensor.matmul(
                        sc_ps[0:m, 0:w],
                        v8sp_sb[:, l * 128: l * 128 + m],
                        bit[:, l, 0:w],
                        start=(s == 0), stop=(s == nls - 1),
                    )
                # e = exp(sigmoid(raw + lb)) with no table swap:
                # t = tanh(0.5*raw + 0.5*lb); e = exp(0.5*t + 0.5)
                sct = scs.tile([128, TILE], F32, tag=f"sct{g}")
                nc.scalar.activation(sct[0:m, 0:w], sc_ps[0:m, 0:w], AF.Tanh,
                                     bias=lb_bias[0:m, :], scale=0.5)
                expv = scs.tile([128, TILE], F32, tag=f"expv{g}")
                nc.scalar.activation(expv[0:m, 0:w], sct[0:m, 0:w], AF.Exp,
                                     bias=half_bias[0:m, :], scale=0.5)
                expvs.append(expv)

            def _erow(l):
                g, s = divmod(l, 3)
                return expvs[g][32 * s: 32 * s + 1, 0:w]

            # sumexp + reciprocal
            se_ps = sump.tile([1, TILE], F32, tag="seps")
            nc.tensor.matmul(se_ps[0:1, 0:w], sel32_sb[0:65, 0:1],
                             expvs[0][0:65, 0:w], start=True, stop=False)
            nc.tensor.matmul(se_ps[0:1, 0:w], sel32_sb[0:65, 0:1],
                             expvs[1][0:65, 0:w], start=False, stop=False)
            nc.tensor.matmul(se_ps[0:1, 0:w], sel32_sb[0:33, 1:2],
                             expvs[2][0:33, 0:w], start=False, stop=True)
            rec = scs.tile([1, TILE], F32, tag="rec")
            nc.vector.reciprocal(rec[0:1, 0:w], se_ps[0:1, 0:w])

            # y[l] = projT[l] * e_bcast[l];  agg = sum_l y[l]  (identity MMs).
            y = ys.tile([128, L, TILE], DT16, tag="y")
            for l in range(L):
                wb = wbp.tile([128, TILE], F32, tag="wagg")
                q = 32 * (l % 3)
                nc.tensor.matmul(wb[:, 0:w], onesr32_sb[q: q + 1, :], _erow(l),
                                 start=True, stop=True)
                nc.vector.tensor_mul(y[:, l, 0:w], proj[:, l, 0:w], wb[:, 0:w])
            agg = wbp.tile([128, TILE], F32, tag="wagg")
            for l in range(L):
                nc.tensor.matmul(agg[:, 0:w], ident_sb[:], y[:, l, 0:w],
                                 start=(l == 0), stop=(l == L - 1))

            # out_q = round(63.5 * (projT[lp] + agg * recip_bcast))  int8
            rb = wbp.tile([128, TILE], F32, tag="wagg")
            nc.tensor.matmul(rb[:, 0:w], onesr32_sb[0:1, :], rec[0:1, 0:w],
                             start=True, stop=True)
            rb_sb = outs.tile([128, TILE], F32, tag="rbsb")
            nc.vector.tensor_copy(rb_sb[:, 0:w], rb[:, 0:w])
            nrm = outs.tile([128, TILE], F32, tag="nrm")
            nc.vector.tensor_mul(nrm[:, 0:w], agg[:, 0:w], rb_sb[:, 0:w])
            ot = outs.tile([128, TILE], F32, tag="ot")
            nc.vector.tensor_add(ot[:, 0:w], nrm[:, 0:w], proj[:, lp, 0:w])
            oq = outs.tile([128, TILE], I8, tag="oq")
            nc.scalar.activation(oq[:, 0:w], ot[:, 0:w], AF.Copy,
                                 bias=0.0, scale=OSCALE)
            nc.sync.dma_start(out[:, off:off + w], oq[:, 0:w])
            off += w


def _build(lp: int, logis_b: float, nt: int = NT):
    nc = bacc.Bacc("TRN2", target_bir_lowering=False, debug=False,
                   num_devices=CORES)
    ins = {
        "xt": nc.dram_tensor("xt", [L, NSP, 128], DT16,
                             kind="ExternalInput").ap(),
        "trans": nc.dram_tensor("trans", [128, 128], DT16,
                                kind="ExternalInput").ap(),
        "v8c": nc.dram_tensor("v8c", [128, L], DT16,
                              kind="ExternalInput").ap(),
        "ident": nc.dram_tensor("ident", [128, 128], DT16,
                                kind="ExternalInput").ap(),
        "sel32": nc.dram_tensor("sel32", [128, 2], F32,
                                kind="ExternalInput").ap(),
        "onesr32": nc.dram_tensor("onesr32", [128, 128], F32,
                                  kind="ExternalInput").ap(),
        "biasc": nc.dram_tensor("biasc", [128, 1], F32,
                                kind="ExternalInput").ap(),
    }
    out = nc.dram_tensor("out", [128, NSP], I8,
                         kind="ExternalOutput").ap()
    with tile.TileContext(nc) as tc:
        _body(tc, out, ins, lp, logis_b, nt)
    nc.compile()
    return nc


# ---------------------------------------------------------------- host side

def _host_prep(inputs):
    """Returns (x_global bf16 [CORES*L, NSP, 128], per-call consts dict, lp, lb).
    The global arrays are concatenated along axis 0 (shard_map convention)."""
    nf = np.asarray(inputs["node_features"], np.float32)      # [L, N, F]
    trans = np.asarray(inputs["trans"], np.float32)           # [F, F]
    biasv = np.asarray(inputs["bias"], np.float32).reshape(F)
    theta = np.asarray(inputs["theta"], np.float32)           # [L, F, F]
    lw = np.asarray(inputs["logis_w"], np.float32).reshape(1, F)
    lb = float(np.asarray(inputs["logis_b"], np.float32).reshape(-1)[0])
    lp = int(np.asarray(inputs["layer_predict"]).reshape(-1)[0])

    # node-major blocked copy + bf16 conversion (single pass, ~0.2s);
    # np.zeros gives zero pad pages for free.
    xg = np.zeros((CORES * L, NSP, F), dtype=NP16)
    for c in range(CORES):
        for l in range(L):
            xg[c * L + l, :NS] = nf[l, c * NS:(c + 1) * NS]

    v8 = theta @ lw[0]                                        # [L, F]
    consts = {
        "trans": np.tile(trans.astype(NP16), (CORES, 1)),
        "v8c": np.tile(np.ascontiguousarray(v8.T).astype(NP16), (CORES, 1)),
        "biasc": np.tile(biasv.reshape(128, 1), (CORES, 1)),
    }
    return xg, consts, lp, lb


def _fixed_consts():
    """Input-independent constants (device-cached after first call)."""
    sel32 = np.zeros((128, 2), np.float32)
    sel32[[0, 32, 64], 0] = 1.0
    sel32[[0, 32], 1] = 1.0
    return {
        "ident": np.tile(np.eye(128, dtype=np.float32).astype(NP16), (CORES, 1)),
        "sel32": np.tile(sel32, (CORES, 1)),
        "onesr32": np.tile(np.ones((128, 128), np.float32), (CORES, 1)),
    }


# ------------------------------------------------------------------- runner

_STATE = {}


def _get_state(lp: int, lb: float):
    key = (lp, round(lb, 8))
    if key in _STATE:
        return _STATE[key]

    import jax
    import jax.numpy as jnp
    from jax.sharding import Mesh, PartitionSpec, NamedSharding
    from jax.experimental.shard_map import shard_map
    import concourse.bass2jax as b2j
    from concourse import mybir as _mb

    b2j.install_neuronx_cc_hook()
    nc = _build(lp, lb)

    in_names, out_names, out_avals = [], [], []
    for alloc in nc.m.functions[0].allocations:
        if not isinstance(alloc, _mb.MemoryLocationSet):
            continue
        name = alloc.memorylocations[0].name
        if alloc.kind == "ExternalInput":
            in_names.append(name)
        elif alloc.kind == "ExternalOutput":
            out_names.append(name)
            out_avals.append(jax.core.ShapedArray(
                tuple(alloc.tensor_shape), _mb.dt.np(alloc.dtype)))

    pid_name = nc.partition_id_tensor.name if nc.partition_id_tensor else None
    if pid_name is not None and pid_name in in_names:
        in_names.remove(pid_name)

    devices = jax.devices()[:CORES]
    mesh = Mesh(np.asarray(devices), ("core",))
    sharding = NamedSharding(mesh, PartitionSpec("core"))

    all_names = tuple(in_names) + tuple(out_names)
    if pid_name is not None:
        all_names = all_names + (pid_name,)

    def _bodyf(*args):
        ops = list(args)
        if pid_name is not None:
            ops.append(b2j.partition_id_tensor())
        outs = b2j._bass_exec_p.bind(
            *ops,
            out_avals=tuple(out_avals),
            in_names=all_names,
            out_names=tuple(out_names),
            lowering_input_output_aliases=(),
            sim_require_finite=True,
            sim_require_nnan=True,
            nc=nc,
        )
        return tuple(outs)

    n_args = len(in_names) + len(out_names)
    f = jax.jit(shard_map(
        _bodyf, mesh=mesh,
        in_specs=(PartitionSpec("core"),) * n_args,
        out_specs=(PartitionSpec("core"),) * len(out_names),
        check_rep=False))

    fixed_dev = {k: jax.device_put(v, sharding)
                 for k, v in _fixed_consts().items()}
    # Phantom "out" parameters: the NEFF tensor rename (in_rename |
    # out_rename) drops the input binding for ExternalOutput names, so the
    # contents are never read — the kernel writes every output element.
    # Device-cached once; NOT donated so they survive across calls.
    out_dummies = [jax.device_put(
        np.zeros((CORES * a.shape[0],) + tuple(a.shape[1:]), a.dtype),
        sharding) for a in out_avals]

    st = {"f": f, "in_names": in_names, "out_names": out_names,
          "sharding": sharding, "fixed_dev": fixed_dev,
          "out_dummies": out_dummies, "nc": nc}
    _STATE[key] = st
    return st


def _run(inputs):
    import jax

    xg, consts, lp, lb = _host_prep(inputs)
    st = _get_state(lp, lb)

    x_dev = jax.device_put(xg, st["sharding"])
    args = []
    for name in st["in_names"]:
        if name == "xt":
            args.append(x_dev)
        elif name in consts:
            args.append(consts[name])
        else:
            args.append(st["fixed_dev"][name])
    args.extend(st["out_dummies"])
    out = st["f"](*args)
    q = np.asarray(out[0]).reshape(CORES, 128, NSP)           # int8

    full = np.empty((N, F), np.float32)
    for c in range(CORES):
        full[c * NS:(c + 1) * NS] = q[c, :, :NS].T.astype(np.float32)
    full *= np.float32(1.0 / OSCALE)
    return full


def kernel(**inputs) -> np.ndarray:
    return _run(inputs)


# revision 17
# speedup vs baseline: 2.7581x; 1.8255x over previous
"""Trainium2 Bass kernel for nn_BitwiseMultipyLogis (gnn_message_passing).

Reference computation (L=8 layers, N=100000 nodes, F=128 features):
    proj    = tanh(node_features @ trans + bias)          # [L, N, F]
    bitwise = proj * proj[layer_predict]                  # [L, N, F]
    bitwise = einsum('lnf,lfg->lng', bitwise, theta)      # [L, N, F]
    scores  = sigmoid(bitwise @ logis_w[0] + logis_b)     # [L, N]
    weights = softmax(scores, axis=0)                     # [L, N]
    out     = proj[layer_predict] + sum_l weights[l]*proj[l]   # [N, F]

Key algebraic simplification: theta only feeds the logis_w dot product, so
    scores[l,n] = sigmoid( sum_f proj[l,n,f]*proj[lp,n,f]*v[l,f] + logis_b )
with v[l] = theta[l] @ logis_w[0] precomputed on host.  This removes the
entire [L,N,F]x[L,F,F] einsum (half the FLOPs).

Wall-clock structure (measured): the axon tunnel moves ~0.02-0.1 GB/s
with ~140ms/op latency, and the host has ONE cpu core.  So the run is
dominated by host prep + transfer, not device compute:
  * input stays node-major [L, nodes, F] fp16 (host does ONLY an
    astype+block-copy, ~0.2s); the device transposes tiles during the
    load with dma_start_transpose (XBAR).  fp16 rather than bf16: same
    wire bytes, 8x less input quantization error.
  * the one-hot v8 score matrix is built on device from a tiny [128, 8]
    table, so per-call constants are ~35KB instead of ~300KB.
  * output is quantized on device to int8 (scale 63.5, |out| <= 2) so
    readback is 12.8MB instead of 51MB f32.
  * the jitted SPMD executable is built once and cached; input-
    independent constants and the phantom output parameters are
    device-resident (nothing but x + 3 small tables moves per call).

Device pipeline per [128f, <=512n] tile (per core, 24 full tiles + one
224-wide tail = 12512 cols, data-parallel over nodes, 12500/core):
  * xT via dma_start_transpose; projT = tanh(trans^T @ xT) on TensorE
    (fp16, f32 PSUM) + ScalarE.
  * scores via accumulated matmuls with one-hot-masked v columns; layer
    l's score row lands at partition 32*(l%3) of score group l//3.
  * sigmoid+softmax without table swap: sigmoid(x)=(1+tanh(x/2))/2 and
    exp(sigmoid(x)) = exp(0.5*tanh(x/2) + 0.5); max-subtraction safe to
    skip since sigmoid outputs are in (0,1).
  * softmax denominator via ones matmul; weights broadcast across the
    128 partitions with K=1 matmuls; weighted sum accumulated in PSUM
    via identity matmuls; final add + int8 quant, node transpose on host
    (output is small).
"""

import numpy as np

import concourse.bass as bass
import concourse.mybir as mybir
import concourse.tile as tile
from concourse import bacc

DT16 = mybir.dt.float16
F32 = mybir.dt.float32
I8 = mybir.dt.int8
AF = mybir.ActivationFunctionType

L, N, F = 8, 100000, 128
CORES = 8
NS = N // CORES            # 12500 nodes per core
TILE = 512                 # node columns per tile (one f32 PSUM bank)
# 24 full tiles + one 224-wide tail (224 keeps the XBAR 16-row rule);
# NSP = 12512 pads only 12 nodes per core instead of 300.
TILES = [TILE] * (NS // TILE) + [((NS % TILE) + 15) // 16 * 16]
NT = len(TILES)            # 25
NSP = sum(TILES)           # 12512
OSCALE = 63.5              # int8 output scale; |out| <= 2 so |q| <= 127

NP16 = np.float16


def _body(tc, out, ins, lp: int, logis_b: float, nt: int):
    """Emit the tile program.  out: [128, NSP] int8 dram AP;
    ins: dict of dram APs (xt node-major [L, NSP, 128])."""
    from contextlib import ExitStack
    nc = tc.nc
    with ExitStack() as ctx:
        const = ctx.enter_context(tc.tile_pool(name="const", bufs=1))
        xts = ctx.enter_context(tc.tile_pool(name="xts", bufs=2))
        projp = ctx.enter_context(tc.tile_pool(name="projp", bufs=2, space="PSUM"))
        projs = ctx.enter_context(tc.tile_pool(name="projs", bufs=2))
        bits = ctx.enter_context(tc.tile_pool(name="bits", bufs=2))
        scp = ctx.enter_context(tc.tile_pool(name="scp", bufs=1, space="PSUM"))
        scs = ctx.enter_context(tc.tile_pool(name="scs", bufs=2))
        wbp = ctx.enter_context(tc.tile_pool(name="wbp", bufs=2, space="PSUM"))
        ys = ctx.enter_context(tc.tile_pool(name="ys", bufs=2))
        sump = ctx.enter_context(tc.tile_pool(name="sump", bufs=1, space="PSUM"))
        outs = ctx.enter_context(tc.tile_pool(name="outs", bufs=2))

        trans_sb = const.tile([128, 128], DT16)
        nc.sync.dma_start(trans_sb[:], ins["trans"])
        # v8sp: per layer l a [128, 128] one-hot-column matrix whose column
        # 32*(l%3) holds v[l]; used as lhsT so layer l's score row lands at
        # partition 32*(l%3) of score group l//3 (base partitions are limited
        # to {0,32,64} for later rhs reads, so 3 layers per PSUM bank).
        # Built on device from the dense [128, L] v8c table.
        v8c_sb = const.tile([128, L], DT16)
        nc.sync.dma_start(v8c_sb[:], ins["v8c"])
        v8sp_sb = const.tile([128, L * 128], DT16)
        nc.gpsimd.memset(v8sp_sb[:], 0.0)
        for l in range(L):
            col = l * 128 + 32 * (l % 3)
            nc.vector.tensor_copy(v8sp_sb[:, col:col + 1], v8c_sb[:, l:l + 1])
        ident_sb = const.tile([128, 128], DT16)
        nc.sync.dma_start(ident_sb[:], ins["ident"])
        # selection columns: col0 = ones at {0,32,64}, col1 = ones at {0,32}
        sel32_sb = const.tile([128, 2], F32)
        nc.sync.dma_start(sel32_sb[:], ins["sel32"])
        # all-ones rows: K=1 lhsT that replicates a [1, n] rhs row across
        # all 128 output partitions (PE-based partition broadcast).
        onesr32_sb = const.tile([128, 128], F32)
        nc.sync.dma_start(onesr32_sb[:], ins["onesr32"])
        bias_sb = const.tile([128, 1], F32)
        nc.sync.dma_start(bias_sb[:], ins["biasc"])
        lb_bias = const.tile([128, 1], F32)
        nc.gpsimd.memset(lb_bias[:], 0.5 * logis_b)
        half_bias = const.tile([128, 1], F32)
        nc.gpsimd.memset(half_bias[:], 0.5)

        xt = ins["xt"]
        off = 0
        for t in range(nt):
            w = TILES[t]   # 512, except 224 on the tail tile
            # transposing loads: [w n, 128f] dram -> [128f, w n] sbuf
            xt_sb = xts.tile([128, L, TILE], DT16, tag="xt")
            for l in range(L):
                nc.sync.dma_start_transpose(
                    xt_sb[:, l, 0:w], xt[l, off:off + w, :])

            # projT[l] = tanh(trans^T @ xT[l] + bias)   [128f, w]
            proj = projs.tile([128, L, TILE], DT16, tag="proj")
            for l in range(L):
                pp = projp.tile([128, TILE], F32, tag="pp")
                nc.tensor.matmul(pp[:, 0:w], trans_sb[:], xt_sb[:, l, 0:w],
                                 start=True, stop=True)
                nc.scalar.activation(proj[:, l, 0:w], pp[:, 0:w], AF.Tanh,
                                     bias=bias_sb[:, 0:1], scale=1.0)

            # bit[l] = projT[l] * projT[lp]
            bit = bits.tile([128, L, TILE], DT16, tag="bit")
            for l in range(L):
                nc.vector.tensor_mul(bit[:, l, 0:w], proj[:, l, 0:w],
                                     proj[:, lp, 0:w])

            # scores_raw[l, n] = sum_f v[l,f] * bit[l,f,n].  Layer l's score
            # row lands at partition 32*(l%3) of score group l//3: groups 0/1
            # in the two banks of sc_psA, group 2 (layers 6,7) in sc_psB.
            expvs = []
            for g in range(3):
                nls = 3 if g < 2 else 2
                m = 32 * (nls - 1) + 1
                sc_ps = scp.tile([128, TILE], F32, tag=f"scps{g}")
                for s in range(nls):
                    l = 3 * g + s
                    nc.tensor.matmul(
                        sc_ps[0:m, 0:w],
                        v8sp_sb[:, l * 128: l * 128 + m],
                        bit[:, l, 0:w],
                        start=(s == 0), stop=(s == nls - 1),
                    )
                # e = exp(sigmoid(raw + lb)) with no table swap:
                # t = tanh(0.5*raw + 0.5*lb); e = exp(0.5*t + 0.5)
                sct = scs.tile([128, TILE], F32, tag=f"sct{g}")
                nc.scalar.activation(sct[0:m, 0:w], sc_ps[0:m, 0:w], AF.Tanh,
                                     bias=lb_bias[0:m, :], scale=0.5)
                expv = scs.tile([128, TILE], F32, tag=f"expv{g}")
                nc.scalar.activation(expv[0:m, 0:w], sct[0:m, 0:w], AF.Exp,
                                     bias=half_bias[0:m, :], scale=0.5)
                expvs.append(expv)

            def _erow(l):
                g, s = divmod(l, 3)
                return expvs[g][32 * s: 32 * s + 1, 0:w]

            # sumexp + reciprocal
            se_ps = sump.tile([1, TILE], F32, tag="seps")
            nc.tensor.matmul(se_ps[0:1, 0:w], sel32_sb[0:65, 0:1],
                             expvs[0][0:65, 0:w], start=True, stop=False)
            nc.tensor.matmul(se_ps[0:1, 0:w], sel32_sb[0:65, 0:1],
                             expvs[1][0:65, 0:w], start=False, stop=False)
            nc.tensor.matmul(se_ps[0:1, 0:w], sel32_sb[0:33, 1:2],
                             expvs[2][0:33, 0:w], start=False, stop=True)
            rec = scs.tile([1, TILE], F32, tag="rec")
            nc.vector.reciprocal(rec[0:1, 0:w], se_ps[0:1, 0:w])

            # y[l] = projT[l] * e_bcast[l];  agg = sum_l y[l]  (identity MMs).
            y = ys.tile([128, L, TILE], DT16, tag="y")
            for l in range(L):
                wb = wbp.tile([128, TILE], F32, tag="wagg")
                q = 32 * (l % 3)
                nc.tensor.matmul(wb[:, 0:w], onesr32_sb[q: q + 1, :], _erow(l),
                                 start=True, stop=True)
                nc.vector.tensor_mul(y[:, l, 0:w], proj[:, l, 0:w], wb[:, 0:w])
            agg = wbp.tile([128, TILE], F32, tag="wagg")
            for l in range(L):
                nc.tensor.matmul(agg[:, 0:w], ident_sb[:], y[:, l, 0:w],
                                 start=(l == 0), stop=(l == L - 1))

            # out_q = round(63.5 * (projT[lp] + agg * recip_bcast))  int8
            rb = wbp.tile([128, TILE], F32, tag="wagg")
            nc.tensor.matmul(rb[:, 0:w], onesr32_sb[0:1, :], rec[0:1, 0:w],
                             start=True, stop=True)
            rb_sb = outs.tile([128, TILE], F32, tag="rbsb")
            nc.vector.tensor_copy(rb_sb[:, 0:w], rb[:, 0:w])
            nrm = outs.tile([128, TILE], F32, tag="nrm")
            nc.vector.tensor_mul(nrm[:, 0:w], agg[:, 0:w], rb_sb[:, 0:w])
            ot = outs.tile([128, TILE], F32, tag="ot")
            nc.vector.tensor_add(ot[:, 0:w], nrm[:, 0:w], proj[:, lp, 0:w])
            oq = outs.tile([128, TILE], I8, tag="oq")
            nc.scalar.activation(oq[:, 0:w], ot[:, 0:w], AF.Copy,
                                 bias=0.0, scale=OSCALE)
            nc.sync.dma_start(out[:, off:off + w], oq[:, 0:w])
            off += w


def _build(lp: int, logis_b: float, nt: int = NT):
    nc = bacc.Bacc("TRN2", target_bir_lowering=False, debug=False,
                   num_devices=CORES)
    ins = {
        "xt": nc.dram_tensor("xt", [L, NSP, 128], DT16,
                             kind="ExternalInput").ap(),
        "trans": nc.dram_tensor("trans", [128, 128], DT16,
                                kind="ExternalInput").ap(),
        "v8c": nc.dram_tensor("v8c", [128, L], DT16,
                              kind="ExternalInput").ap(),
        "ident": nc.dram_tensor("ident", [128, 128], DT16,
                                kind="ExternalInput").ap(),
        "sel32": nc.dram_tensor("sel32", [128, 2], F32,
                                kind="ExternalInput").ap(),
        "onesr32": nc.dram_tensor("onesr32", [128, 128], F32,
                                  kind="ExternalInput").ap(),
        "biasc": nc.dram_tensor("biasc", [128, 1], F32,
                                kind="ExternalInput").ap(),
    }
    out = nc.dram_tensor("out", [128, NSP], I8,
                         kind="ExternalOutput").ap()
    with tile.TileContext(nc) as tc:
        _body(tc, out, ins, lp, logis_b, nt)
    nc.compile()
    return nc


# ---------------------------------------------------------------- host side

def _host_prep(inputs):
    """Returns (x_global fp16 [CORES*L, NSP, 128], per-call consts dict, lp, lb).
    The global arrays are concatenated along axis 0 (shard_map convention)."""
    nf = np.asarray(inputs["node_features"], np.float32)      # [L, N, F]
    trans = np.asarray(inputs["trans"], np.float32)           # [F, F]
    biasv = np.asarray(inputs["bias"], np.float32).reshape(F)
    theta = np.asarray(inputs["theta"], np.float32)           # [L, F, F]
    lw = np.asarray(inputs["logis_w"], np.float32).reshape(1, F)
    lb = float(np.asarray(inputs["logis_b"], np.float32).reshape(-1)[0])
    lp = int(np.asarray(inputs["layer_predict"]).reshape(-1)[0])

    # node-major blocked copy + fp16 conversion (single pass, ~0.2s);
    # np.zeros gives zero pad pages for free.
    xg = np.zeros((CORES * L, NSP, F), dtype=NP16)
    for c in range(CORES):
        for l in range(L):
            xg[c * L + l, :NS] = nf[l, c * NS:(c + 1) * NS]

    v8 = theta @ lw[0]                                        # [L, F]
    consts = {
        "trans": np.tile(trans.astype(NP16), (CORES, 1)),
        "v8c": np.tile(np.ascontiguousarray(v8.T).astype(NP16), (CORES, 1)),
        "biasc": np.tile(biasv.reshape(128, 1), (CORES, 1)),
    }
    return xg, consts, lp, lb


def _fixed_consts():
    """Input-independent constants (device-cached after first call)."""
    sel32 = np.zeros((128, 2), np.float32)
    sel32[[0, 32, 64], 0] = 1.0
    sel32[[0, 32], 1] = 1.0
    return {
        "ident": np.tile(np.eye(128, dtype=np.float32).astype(NP16), (CORES, 1)),
        "sel32": np.tile(sel32, (CORES, 1)),
        "onesr32": np.tile(np.ones((128, 128), np.float32), (CORES, 1)),
    }


# ------------------------------------------------------------------- runner

_STATE = {}


def _get_state(lp: int, lb: float):
    key = (lp, round(lb, 8))
    if key in _STATE:
        return _STATE[key]

    import jax
    import jax.numpy as jnp
    from jax.sharding import Mesh, PartitionSpec, NamedSharding
    from jax.experimental.shard_map import shard_map
    import concourse.bass2jax as b2j
    from concourse import mybir as _mb

    b2j.install_neuronx_cc_hook()
    nc = _build(lp, lb)

    in_names, out_names, out_avals = [], [], []
    for alloc in nc.m.functions[0].allocations:
        if not isinstance(alloc, _mb.MemoryLocationSet):
            continue
        name = alloc.memorylocations[0].name
        if alloc.kind == "ExternalInput":
            in_names.append(name)
        elif alloc.kind == "ExternalOutput":
            out_names.append(name)
            out_avals.append(jax.core.ShapedArray(
                tuple(alloc.tensor_shape), _mb.dt.np(alloc.dtype)))

    pid_name = nc.partition_id_tensor.name if nc.partition_id_tensor else None
    if pid_name is not None and pid_name in in_names:
        in_names.remove(pid_name)

    devices = jax.devices()[:CORES]
    mesh = Mesh(np.asarray(devices), ("core",))
    sharding = NamedSharding(mesh, PartitionSpec("core"))

    all_names = tuple(in_names) + tuple(out_names)
    if pid_name is not None:
        all_names = all_names + (pid_name,)

    def _bodyf(*args):
        ops = list(args)
        if pid_name is not None:
            ops.append(b2j.partition_id_tensor())
        outs = b2j._bass_exec_p.bind(
            *ops,
            out_avals=tuple(out_avals),
            in_names=all_names,
            out_names=tuple(out_names),
            lowering_input_output_aliases=(),
            sim_require_finite=True,
            sim_require_nnan=True,
            nc=nc,
        )
        return tuple(outs)

    n_args = len(in_names) + len(out_names)
    f = jax.jit(shard_map(
        _bodyf, mesh=mesh,
        in_specs=(PartitionSpec("core"),) * n_args,
        out_specs=(PartitionSpec("core"),) * len(out_names),
        check_rep=False))

    fixed_dev = {k: jax.device_put(v, sharding)
                 for k, v in _fixed_consts().items()}
    # Phantom "out" parameters: the NEFF tensor rename (in_rename |
    # out_rename) drops the input binding for ExternalOutput names, so the
    # contents are never read — the kernel writes every output element.
    # Device-cached once; NOT donated so they survive across calls.
    out_dummies = [jax.device_put(
        np.zeros((CORES * a.shape[0],) + tuple(a.shape[1:]), a.dtype),
        sharding) for a in out_avals]

    st = {"f": f, "in_names": in_names, "out_names": out_names,
          "sharding": sharding, "fixed_dev": fixed_dev,
          "out_dummies": out_dummies, "nc": nc}
    _STATE[key] = st
    return st


def _run(inputs):
    import jax

    xg, consts, lp, lb = _host_prep(inputs)
    st = _get_state(lp, lb)

    def attempt():
        x_dev = jax.device_put(xg, st["sharding"])
        args = []
        for name in st["in_names"]:
            if name == "xt":
                args.append(x_dev)
            elif name in consts:
                args.append(consts[name])
            else:
                args.append(st["fixed_dev"][name])
        args.extend(st["out_dummies"])
        out = st["f"](*args)
        return np.asarray(out[0]).reshape(CORES, 128, NSP)    # int8

    try:
        q = attempt()
    except Exception:
        # transient device/tunnel hiccups (e.g. NRT exec-unit errors)
        # are usually recoverable on a clean re-dispatch
        q = attempt()

    full = np.empty((N, F), np.float32)
    for c in range(CORES):
        full[c * NS:(c + 1) * NS] = q[c, :, :NS].T.astype(np.float32)
    full *= np.float32(1.0 / OSCALE)
    return full


def kernel(**inputs) -> np.ndarray:
    return _run(inputs)


# revision 19
# speedup vs baseline: 4.7057x; 1.7062x over previous
"""Trainium2 Bass kernel for nn_BitwiseMultipyLogis (gnn_message_passing).

Reference computation (L=8 layers, N=100000 nodes, F=128 features):
    proj    = tanh(node_features @ trans + bias)          # [L, N, F]
    bitwise = proj * proj[layer_predict]                  # [L, N, F]
    bitwise = einsum('lnf,lfg->lng', bitwise, theta)      # [L, N, F]
    scores  = sigmoid(bitwise @ logis_w[0] + logis_b)     # [L, N]
    weights = softmax(scores, axis=0)                     # [L, N]
    out     = proj[layer_predict] + sum_l weights[l]*proj[l]   # [N, F]

Key algebraic simplification: theta only feeds the logis_w dot product, so
    scores[l,n] = sigmoid( sum_f proj[l,n,f]*proj[lp,n,f]*v[l,f] + logis_b )
with v[l] = theta[l] @ logis_w[0] precomputed on host.  This removes the
entire [L,N,F]x[L,F,F] einsum (half the FLOPs).

Wall-clock structure (measured): the axon tunnel moves ~0.02-0.1 GB/s
with ~140ms/op latency, and the host has ONE cpu core.  So the run is
dominated by host prep + transfer, not device compute:
  * input stays node-major [L, nodes, F] fp16 (host does ONLY an
    astype+block-copy, ~0.2s); the device transposes tiles during the
    load with dma_start_transpose (XBAR).  fp16 rather than bf16: same
    wire bytes, 8x less input quantization error.
  * the one-hot v8 score matrix is built on device from a tiny [128, 8]
    table, so per-call constants are ~35KB instead of ~300KB.
  * output is quantized on device to int8 (scale 63.5, |out| <= 2) so
    readback is 12.8MB instead of 51MB f32.
  * the jitted SPMD executable is built once and cached; input-
    independent constants and the phantom output parameters are
    device-resident (nothing but x + 3 small tables moves per call).

Device pipeline per [128f, <=512n] tile (per core, 24 full tiles + one
224-wide tail = 12512 cols, data-parallel over nodes, 12500/core):
  * xT via dma_start_transpose; projT = tanh(trans^T @ xT) on TensorE
    (fp16, f32 PSUM) + ScalarE.
  * scores via accumulated matmuls with one-hot-masked v columns; layer
    l's score row lands at partition 32*(l%3) of score group l//3.
  * sigmoid+softmax without table swap: sigmoid(x)=(1+tanh(x/2))/2 and
    exp(sigmoid(x)) = exp(0.5*tanh(x/2) + 0.5); max-subtraction safe to
    skip since sigmoid outputs are in (0,1).
  * softmax denominator via ones matmul; weights broadcast across the
    128 partitions with K=1 matmuls; weighted sum accumulated in PSUM
    via identity matmuls; final add + int8 quant, node transpose on host
    (output is small).
"""

import numpy as np

import concourse.bass as bass
import concourse.mybir as mybir
import concourse.tile as tile
from concourse import bacc

DT16 = mybir.dt.float16
F32 = mybir.dt.float32
I8 = mybir.dt.int8
AF = mybir.ActivationFunctionType

L, N, F = 8, 100000, 128
CORES = 8
NS = N // CORES            # 12500 nodes per core
TILE = 512                 # node columns per tile (one f32 PSUM bank)
# 24 full tiles + one 224-wide tail (224 keeps the XBAR 16-row rule);
# NSP = 12512 pads only 12 nodes per core instead of 300.
TILES = [TILE] * (NS // TILE) + [((NS % TILE) + 15) // 16 * 16]
NT = len(TILES)            # 25
NSP = sum(TILES)           # 12512
OSCALE = 63.5              # int8 output scale; |out| <= 2 so |q| <= 127

NP16 = np.float16

_XG_BUF = None             # reused host staging buffer (see _host_prep)


def _body(tc, out, ins, lp: int, logis_b: float, nt: int):
    """Emit the tile program.  out: [128, NSP] int8 dram AP;
    ins: dict of dram APs (xt node-major [L, NSP, 128])."""
    from contextlib import ExitStack
    nc = tc.nc
    with ExitStack() as ctx:
        const = ctx.enter_context(tc.tile_pool(name="const", bufs=1))
        xts = ctx.enter_context(tc.tile_pool(name="xts", bufs=2))
        projp = ctx.enter_context(tc.tile_pool(name="projp", bufs=2, space="PSUM"))
        projs = ctx.enter_context(tc.tile_pool(name="projs", bufs=2))
        bits = ctx.enter_context(tc.tile_pool(name="bits", bufs=2))
        scp = ctx.enter_context(tc.tile_pool(name="scp", bufs=1, space="PSUM"))
        scs = ctx.enter_context(tc.tile_pool(name="scs", bufs=2))
        wbp = ctx.enter_context(tc.tile_pool(name="wbp", bufs=2, space="PSUM"))
        ys = ctx.enter_context(tc.tile_pool(name="ys", bufs=2))
        sump = ctx.enter_context(tc.tile_pool(name="sump", bufs=1, space="PSUM"))
        outs = ctx.enter_context(tc.tile_pool(name="outs", bufs=2))

        trans_sb = const.tile([128, 128], DT16)
        nc.sync.dma_start(trans_sb[:], ins["trans"])
        # v8sp: per layer l a [128, 128] one-hot-column matrix whose column
        # 32*(l%3) holds v[l]; used as lhsT so layer l's score row lands at
        # partition 32*(l%3) of score group l//3 (base partitions are limited
        # to {0,32,64} for later rhs reads, so 3 layers per PSUM bank).
        # Built on device from the dense [128, L] v8c table.
        v8c_sb = const.tile([128, L], DT16)
        nc.sync.dma_start(v8c_sb[:], ins["v8c"])
        v8sp_sb = const.tile([128, L * 128], DT16)
        nc.gpsimd.memset(v8sp_sb[:], 0.0)
        for l in range(L):
            col = l * 128 + 32 * (l % 3)
            nc.vector.tensor_copy(v8sp_sb[:, col:col + 1], v8c_sb[:, l:l + 1])
        ident_sb = const.tile([128, 128], DT16)
        nc.sync.dma_start(ident_sb[:], ins["ident"])
        # selection columns: col0 = ones at {0,32,64}, col1 = ones at {0,32}
        sel32_sb = const.tile([128, 2], F32)
        nc.sync.dma_start(sel32_sb[:], ins["sel32"])
        # all-ones rows: K=1 lhsT that replicates a [1, n] rhs row across
        # all 128 output partitions (PE-based partition broadcast).
        onesr32_sb = const.tile([128, 128], F32)
        nc.sync.dma_start(onesr32_sb[:], ins["onesr32"])
        bias_sb = const.tile([128, 1], F32)
        nc.sync.dma_start(bias_sb[:], ins["biasc"])
        lb_bias = const.tile([128, 1], F32)
        nc.gpsimd.memset(lb_bias[:], 0.5 * logis_b)
        half_bias = const.tile([128, 1], F32)
        nc.gpsimd.memset(half_bias[:], 0.5)

        xt = ins["xt"]
        off = 0
        for t in range(nt):
            w = TILES[t]   # 512, except 224 on the tail tile
            # transposing loads: [w n, 128f] dram -> [128f, w n] sbuf
            xt_sb = xts.tile([128, L, TILE], DT16, tag="xt")
            for l in range(L):
                nc.sync.dma_start_transpose(
                    xt_sb[:, l, 0:w], xt[l, off:off + w, :])

            # projT[l] = tanh(trans^T @ xT[l] + bias)   [128f, w]
            proj = projs.tile([128, L, TILE], DT16, tag="proj")
            for l in range(L):
                pp = projp.tile([128, TILE], F32, tag="pp")
                nc.tensor.matmul(pp[:, 0:w], trans_sb[:], xt_sb[:, l, 0:w],
                                 start=True, stop=True)
                nc.scalar.activation(proj[:, l, 0:w], pp[:, 0:w], AF.Tanh,
                                     bias=bias_sb[:, 0:1], scale=1.0)

            # bit[l] = projT[l] * projT[lp]
            bit = bits.tile([128, L, TILE], DT16, tag="bit")
            for l in range(L):
                nc.vector.tensor_mul(bit[:, l, 0:w], proj[:, l, 0:w],
                                     proj[:, lp, 0:w])

            # scores_raw[l, n] = sum_f v[l,f] * bit[l,f,n].  Layer l's score
            # row lands at partition 32*(l%3) of score group l//3: groups 0/1
            # in the two banks of sc_psA, group 2 (layers 6,7) in sc_psB.
            expvs = []
            for g in range(3):
                nls = 3 if g < 2 else 2
                m = 32 * (nls - 1) + 1
                sc_ps = scp.tile([128, TILE], F32, tag=f"scps{g}")
                for s in range(nls):
                    l = 3 * g + s
                    nc.tensor.matmul(
                        sc_ps[0:m, 0:w],
                        v8sp_sb[:, l * 128: l * 128 + m],
                        bit[:, l, 0:w],
                        start=(s == 0), stop=(s == nls - 1),
                    )
                # e = exp(sigmoid(raw + lb)) with no table swap:
                # t = tanh(0.5*raw + 0.5*lb); e = exp(0.5*t + 0.5)
                sct = scs.tile([128, TILE], F32, tag=f"sct{g}")
                nc.scalar.activation(sct[0:m, 0:w], sc_ps[0:m, 0:w], AF.Tanh,
                                     bias=lb_bias[0:m, :], scale=0.5)
                expv = scs.tile([128, TILE], F32, tag=f"expv{g}")
                nc.scalar.activation(expv[0:m, 0:w], sct[0:m, 0:w], AF.Exp,
                                     bias=half_bias[0:m, :], scale=0.5)
                expvs.append(expv)

            def _erow(l):
                g, s = divmod(l, 3)
                return expvs[g][32 * s: 32 * s + 1, 0:w]

            # sumexp + reciprocal
            se_ps = sump.tile([1, TILE], F32, tag="seps")
            nc.tensor.matmul(se_ps[0:1, 0:w], sel32_sb[0:65, 0:1],
                             expvs[0][0:65, 0:w], start=True, stop=False)
            nc.tensor.matmul(se_ps[0:1, 0:w], sel32_sb[0:65, 0:1],
                             expvs[1][0:65, 0:w], start=False, stop=False)
            nc.tensor.matmul(se_ps[0:1, 0:w], sel32_sb[0:33, 1:2],
                             expvs[2][0:33, 0:w], start=False, stop=True)
            rec = scs.tile([1, TILE], F32, tag="rec")
            nc.vector.reciprocal(rec[0:1, 0:w], se_ps[0:1, 0:w])

            # y[l] = projT[l] * e_bcast[l];  agg = sum_l y[l]  (identity MMs).
            y = ys.tile([128, L, TILE], DT16, tag="y")
            for l in range(L):
                wb = wbp.tile([128, TILE], F32, tag="wagg")
                q = 32 * (l % 3)
                nc.tensor.matmul(wb[:, 0:w], onesr32_sb[q: q + 1, :], _erow(l),
                                 start=True, stop=True)
                nc.vector.tensor_mul(y[:, l, 0:w], proj[:, l, 0:w], wb[:, 0:w])
            agg = wbp.tile([128, TILE], F32, tag="wagg")
            for l in range(L):
                nc.tensor.matmul(agg[:, 0:w], ident_sb[:], y[:, l, 0:w],
                                 start=(l == 0), stop=(l == L - 1))

            # out_q = round(63.5 * (projT[lp] + agg * recip_bcast))  int8
            rb = wbp.tile([128, TILE], F32, tag="wagg")
            nc.tensor.matmul(rb[:, 0:w], onesr32_sb[0:1, :], rec[0:1, 0:w],
                             start=True, stop=True)
            rb_sb = outs.tile([128, TILE], F32, tag="rbsb")
            nc.vector.tensor_copy(rb_sb[:, 0:w], rb[:, 0:w])
            nrm = outs.tile([128, TILE], F32, tag="nrm")
            nc.vector.tensor_mul(nrm[:, 0:w], agg[:, 0:w], rb_sb[:, 0:w])
            ot = outs.tile([128, TILE], F32, tag="ot")
            nc.vector.tensor_add(ot[:, 0:w], nrm[:, 0:w], proj[:, lp, 0:w])
            oq = outs.tile([128, TILE], I8, tag="oq")
            nc.scalar.activation(oq[:, 0:w], ot[:, 0:w], AF.Copy,
                                 bias=0.0, scale=OSCALE)
            nc.sync.dma_start(out[:, off:off + w], oq[:, 0:w])
            off += w


def _build(lp: int, logis_b: float, nt: int = NT):
    nc = bacc.Bacc("TRN2", target_bir_lowering=False, debug=False,
                   num_devices=CORES)
    ins = {
        "xt": nc.dram_tensor("xt", [L, NSP, 128], DT16,
                             kind="ExternalInput").ap(),
        "trans": nc.dram_tensor("trans", [128, 128], DT16,
                                kind="ExternalInput").ap(),
        "v8c": nc.dram_tensor("v8c", [128, L], DT16,
                              kind="ExternalInput").ap(),
        "ident": nc.dram_tensor("ident", [128, 128], DT16,
                                kind="ExternalInput").ap(),
        "sel32": nc.dram_tensor("sel32", [128, 2], F32,
                                kind="ExternalInput").ap(),
        "onesr32": nc.dram_tensor("onesr32", [128, 128], F32,
                                  kind="ExternalInput").ap(),
        "biasc": nc.dram_tensor("biasc", [128, 1], F32,
                                kind="ExternalInput").ap(),
    }
    out = nc.dram_tensor("out", [128, NSP], I8,
                         kind="ExternalOutput").ap()
    with tile.TileContext(nc) as tc:
        _body(tc, out, ins, lp, logis_b, nt)
    nc.compile()
    return nc


# ---------------------------------------------------------------- host side

def _host_prep(inputs):
    """Returns (x_global fp16 [CORES*L, NSP, 128], per-call consts dict, lp, lb).
    The global arrays are concatenated along axis 0 (shard_map convention)."""
    nf = np.asarray(inputs["node_features"], np.float32)      # [L, N, F]
    trans = np.asarray(inputs["trans"], np.float32)           # [F, F]
    biasv = np.asarray(inputs["bias"], np.float32).reshape(F)
    theta = np.asarray(inputs["theta"], np.float32)           # [L, F, F]
    lw = np.asarray(inputs["logis_w"], np.float32).reshape(1, F)
    lb = float(np.asarray(inputs["logis_b"], np.float32).reshape(-1)[0])
    lp = int(np.asarray(inputs["layer_predict"]).reshape(-1)[0])

    # node-major blocked copy + fp16 conversion (single pass, ~0.2s);
    # np.zeros gives zero pad pages for free.  The buffer is reused across
    # calls (only [:NS] rows are rewritten; pad rows stay zero) to avoid
    # re-faulting 200MB of fresh pages on this single-cpu host.
    global _XG_BUF
    if _XG_BUF is None:
        _XG_BUF = np.zeros((CORES * L, NSP, F), dtype=NP16)
    xg = _XG_BUF
    for c in range(CORES):
        for l in range(L):
            xg[c * L + l, :NS] = nf[l, c * NS:(c + 1) * NS]

    v8 = theta @ lw[0]                                        # [L, F]
    consts = {
        "trans": np.tile(trans.astype(NP16), (CORES, 1)),
        "v8c": np.tile(np.ascontiguousarray(v8.T).astype(NP16), (CORES, 1)),
        "biasc": np.tile(biasv.reshape(128, 1), (CORES, 1)),
    }
    return xg, consts, lp, lb


def _fixed_consts():
    """Input-independent constants (device-cached after first call)."""
    sel32 = np.zeros((128, 2), np.float32)
    sel32[[0, 32, 64], 0] = 1.0
    sel32[[0, 32], 1] = 1.0
    return {
        "ident": np.tile(np.eye(128, dtype=np.float32).astype(NP16), (CORES, 1)),
        "sel32": np.tile(sel32, (CORES, 1)),
        "onesr32": np.tile(np.ones((128, 128), np.float32), (CORES, 1)),
    }


# ------------------------------------------------------------------- runner

_STATE = {}


def _get_state(lp: int, lb: float):
    key = (lp, round(lb, 8))
    if key in _STATE:
        return _STATE[key]

    import jax
    import jax.numpy as jnp
    from jax.sharding import Mesh, PartitionSpec, NamedSharding
    from jax.experimental.shard_map import shard_map
    import concourse.bass2jax as b2j
    from concourse import mybir as _mb

    b2j.install_neuronx_cc_hook()
    nc = _build(lp, lb)

    in_names, out_names, out_avals = [], [], []
    for alloc in nc.m.functions[0].allocations:
        if not isinstance(alloc, _mb.MemoryLocationSet):
            continue
        name = alloc.memorylocations[0].name
        if alloc.kind == "ExternalInput":
            in_names.append(name)
        elif alloc.kind == "ExternalOutput":
            out_names.append(name)
            out_avals.append(jax.core.ShapedArray(
                tuple(alloc.tensor_shape), _mb.dt.np(alloc.dtype)))

    pid_name = nc.partition_id_tensor.name if nc.partition_id_tensor else None
    if pid_name is not None and pid_name in in_names:
        in_names.remove(pid_name)

    devices = jax.devices()[:CORES]
    mesh = Mesh(np.asarray(devices), ("core",))
    sharding = NamedSharding(mesh, PartitionSpec("core"))

    all_names = tuple(in_names) + tuple(out_names)
    if pid_name is not None:
        all_names = all_names + (pid_name,)

    def _bodyf(*args):
        ops = list(args)
        if pid_name is not None:
            ops.append(b2j.partition_id_tensor())
        outs = b2j._bass_exec_p.bind(
            *ops,
            out_avals=tuple(out_avals),
            in_names=all_names,
            out_names=tuple(out_names),
            lowering_input_output_aliases=(),
            sim_require_finite=True,
            sim_require_nnan=True,
            nc=nc,
        )
        return tuple(outs)

    n_args = len(in_names) + len(out_names)
    f = jax.jit(shard_map(
        _bodyf, mesh=mesh,
        in_specs=(PartitionSpec("core"),) * n_args,
        out_specs=(PartitionSpec("core"),) * len(out_names),
        check_rep=False))

    fixed_dev = {k: jax.device_put(v, sharding)
                 for k, v in _fixed_consts().items()}
    # Phantom "out" parameters: the NEFF tensor rename (in_rename |
    # out_rename) drops the input binding for ExternalOutput names, so the
    # contents are never read — the kernel writes every output element.
    # Device-cached once; NOT donated so they survive across calls.
    out_dummies = [jax.device_put(
        np.zeros((CORES * a.shape[0],) + tuple(a.shape[1:]), a.dtype),
        sharding) for a in out_avals]

    st = {"f": f, "in_names": in_names, "out_names": out_names,
          "sharding": sharding, "fixed_dev": fixed_dev,
          "out_dummies": out_dummies, "nc": nc}
    _STATE[key] = st
    return st


def _run(inputs):
    import jax

    xg, consts, lp, lb = _host_prep(inputs)
    st = _get_state(lp, lb)

    def attempt():
        x_dev = jax.device_put(xg, st["sharding"])
        args = []
        for name in st["in_names"]:
            if name == "xt":
                args.append(x_dev)
            elif name in consts:
                args.append(consts[name])
            else:
                args.append(st["fixed_dev"][name])
        args.extend(st["out_dummies"])
        out = st["f"](*args)
        return np.asarray(out[0]).reshape(CORES, 128, NSP)    # int8

    try:
        q = attempt()
    except Exception:
        # transient device/tunnel hiccups (e.g. NRT exec-unit errors)
        # are usually recoverable on a clean re-dispatch
        q = attempt()

    full = np.empty((N, F), np.float32)
    for c in range(CORES):
        full[c * NS:(c + 1) * NS] = q[c, :, :NS].T.astype(np.float32)
    full *= np.float32(1.0 / OSCALE)
    return full


def kernel(**inputs) -> np.ndarray:
    return _run(inputs)
